# revision 55
# baseline (speedup 1.0000x reference)
"""Trainium2 Bass kernel: conv3d(16ch,3x3x3,VALID) -> channel softmax -> 2x maxpool3d(2).

Full inputs: x [8,3,96,96,96] f32, w [16,3,3,3,3] f32, b [16] f32.
Output: [8,16,23,23,23] f32.  Data-parallel: 1 sample per NeuronCore.

Per-core pipeline (sample x_i [3,96,96,96] -> out_i [16,23,23,23]):
  Only conv outputs d,h,w in [0,92) survive the two maxpools (23*4=92).
  Partition layout: p = 16*g + c for 8 h-quads g and 16 channels c.
  Free layout per conv depth d: (hl 4, w 92) -> N=368 per matmul.

  conv: single-term fp16 (x, w both rounded to fp16; end-to-end max err
  1.1e-2 vs the 2e-2 gate, dominated by the f16 lnS below). lhsT [108,64]
  holds 4 diagonal [27,16] blocks, row r = 27g + 9kd + 3ci + kh; kw is
  handled by 3 PSUM-accumulated matmuls on column-shifted rhs views. Two
  col-group matmuls (tile cols 0-63 / 64-127) cover 8 quads. h-chunks
  (0,8,16): the last has 7 quads and relies on lhsT[0:81,:,0:64] having
  zero cols 48-63 so partitions 112-127 get zeros, not stale PSUM.

  softmax+pool in log domain: y = (z+b) - ln(lambda*S), and
  maxpool(softmax) = exp(maxpool(y)) since exp is monotone.
    exp:  ACT e = exp(z + b) -> SBUF bf16
    sum:  PE lhsT ws32 [128,32] bf16 (lambda=2^-38 group indicators,
          cols duplicated) -> S at PSUM rows 32jl..32jl+32, 2 d-slices
          per [128,512] PSUM tile (per half-d-quad normalization keeps
          the dependency chain short so PE never starves).
    ln:   one ACT Ln->f16 over the packed [64,368] tile = lnS rounded
          to f16 (prec='f16'); prec='hilo' adds Ln->f32 + a DVE subtract
          writing lo = ln - hi in place over rows 32jl..32jl+8 (the
          duplicated-S rows make that legal at 32-aligned bases).
    sub:  one K=8 (f16) or K=16 (hilo) matmul per slice accumulates
          -lnS onto the logits PSUM (lhsT bc: -1 group indicators).
    pool: DVE reduce_max over (hl,wl) per slice -> hp[:,j,:], then over
          the d-quad -> fin [128,23]
    out:  ACT exp(fin + b - 38*ln2), one DMA per (dq, chunk) via SWDGE
          (gpsimd) so a waiting store never blocks rhs loads queued on
          the sync/scalar sequencers.

  Cost-model-guided choices (TimelineSim is the graded metric and tracks
  HW within ~20%): matmul cost = out-free-size x 1cyc/row (f16), so the
  kernel minimizes total streamed rows (1-term f16 conv = 3x fewer than
  the old hi/lo); an explicit LoadActFuncSet of natural_log_exp_and_others
  up front stops the per-Exp/Ln table reload insertion (~355us saved);
  DMA issue is spread across sync/scalar/gpsimd DGE paths.

  DMA: x is staged once (9 shifted HBM->HBM copies per d-chunk, 3 chunks,
  3x bytes) into B9_k[3ci+kh, d, j] = x[ci, 32k+d, kh*96+j]; kd becomes a
  free-dim offset so each rhs im2col tile loads as 12 nine-partition DMAs
  (innermost 768B, full DMA rate). Staging pieces are interleaved between
  rhs-load blocks so the DMA FIFO never sees a long staging burst.

  Scheduling details that each bought 10+us: stage-0's staging DMAs are
  split by j-range (Tile dependency tracking is range-level, so the first
  block's rhs loads only wait for the j<3456 bytes they read); conv
  matmuls are ordered a-outer/kw-inner so each unit starts on rhs tile
  a=0 while a=1 is still loading; bcast+pool emission trails the convs by
  one half-d-quad (deeper pipelining loses: PSUM has exactly 8 banks =
  6 logit tiles + 2 sum tiles).
"""

import numpy as np
from contextlib import ExitStack

import concourse.bass as bass
import concourse.bacc as bacc
import concourse.tile as tile
from concourse import mybir
from concourse.bass_utils import run_bass_kernel_spmd

F32 = mybir.dt.float32
BF16 = mybir.dt.bfloat16
F16 = mybir.dt.float16

N_CORES = 8
CIN, S = 3, 96
COUT = 16
Q = 23          # pooled output size per dim
DU = 92         # conv positions used per dim (23*4)
S2 = S * S      # 9216
S3 = S * S * S
HW27 = 93 * 96  # 8928: flattened (h,w) span per staged slot
BD = 94         # staged depth extent (unused in B9 path; kept for reference)

_cache: dict = {}


def _mk_tail(nc, prec, bct, biast2, out_, hp, Ls, hl16, jh, dq, hq0, nq,
             finp, outp):
    """Deferred bcast + pools (+ dq finalization) for one half-d-quad."""
    Q_ = Q

    def fire():
        kb = 16 if prec == 'hilo' else 8
        for jl in range(2):
            j = 2 * jh + jl
            # --- y = z - ln(lambda*S): K<=16 accumulate matmul ---
            nc.tensor.matmul(
                out=Ls[jl][:, 0:368],
                lhsT=bct[32 * jl:32 * jl + kb, :],
                rhs=hl16[32 * jl:32 * jl + kb, :],
                start=False, stop=True,
                skip_group_check=True,
                tile_position=(32 * jl, 0),
            )
            # --- maxpool over (hl, wl) -> hp[:, j, :] ---
            nc.vector.reduce_max(
                out=hp[:, j, :],
                in_=Ls[jl][:, 0:368].rearrange(
                    "p (hl wq wl) -> p wq hl wl", hl=4, wq=Q_),
                axis=mybir.AxisListType.XY,
            )
        if jh == 1:
            # --- maxpool over d, back to prob domain, store ---
            fin = finp.tile([128, Q_], F32)
            nc.vector.reduce_max(
                out=fin,
                in_=hp.rearrange("p j w -> p w j"),
                axis=mybir.AxisListType.X,
            )
            ot = outp.tile([128, Q_], F32)
            nc.scalar.activation(
                out=ot, in_=fin,
                func=mybir.ActivationFunctionType.Exp,
                bias=biast2[:, 0:1],
            )
            # out-DMA via SWDGE (gpsimd): a sync/scalar-queued store would
            # hold that SEQ while waiting on this dq's compute, blocking
            # the next block's rhs loads queued behind it.  The final
            # block's stores go via sync instead: no rhs loads remain to
            # block, HWDGE is idle, and its path is ~0.5us shorter, which
            # trims the flush tail.
            eng = nc.sync if (hq0 == 16 and dq >= 16) else nc.gpsimd
            eng.dma_start(
                out=out_[:][:, dq, hq0:hq0 + nq, :].rearrange(
                    "c g w -> g c w"),
                in_=ot[0:16 * nq, :],
            )
    return fire


def _emit(nc, xh, wl_, ws_, bc_, bias_, bias2_, out_, stage='full', prec='hilo'):
    # Preload the one ACT table set containing BOTH Exp and Ln; the
    # insert_act_table_loads pass then sees every activation covered and
    # emits no per-switch reloads (saves ~1.3us x ~276 switches).
    from concourse.hw_specs import get_activation_tables
    _tabs = list(get_activation_tables(nc.m.arch).keys())
    nc.scalar.add_instruction(mybir.InstLoadActFuncSet(
        name="preload_act",
        act_func_set_id=_tabs.index("natural_log_exp_and_others"),
        ins=[], outs=[]))
    with tile.TileContext(nc) as tc, ExitStack() as ctx:
        consts = ctx.enter_context(tc.tile_pool(name="consts", bufs=1))
        rhsp = ctx.enter_context(tc.tile_pool(name="rhs", bufs=3))
        ep = ctx.enter_context(tc.tile_pool(name="e", bufs=6))
        hlp = ctx.enter_context(tc.tile_pool(name="hl", bufs=2))
        lnp = ctx.enter_context(tc.tile_pool(name="ln", bufs=2))
        hpp = ctx.enter_context(tc.tile_pool(name="hp", bufs=2))
        finp = ctx.enter_context(tc.tile_pool(name="fin", bufs=2))
        outp = ctx.enter_context(tc.tile_pool(name="outt", bufs=2))
        psl = ctx.enter_context(tc.tile_pool(name="psl", bufs=6, space="PSUM"))
        pss = ctx.enter_context(tc.tile_pool(name="pss", bufs=2, space="PSUM"))
        dramp = ctx.enter_context(tc.tile_pool(name="dram", bufs=1, space="DRAM"))

        # constants: only wlt is needed by the first matmul; the rest are
        # loaded after stage-0 + the first rhs loads enter the DMA FIFO
        wlt = consts.tile([108, 3, 64], F16, tag="wl")
        nc.sync.dma_start(out=wlt, in_=wl_[:])
        wst = consts.tile([128, 32], BF16, tag="ws")
        bct = consts.tile([128, 128], F16, tag="bc")
        biast = consts.tile([128, 1], F32, tag="bias")
        biast2 = consts.tile([128, 1], F32, tag="bias2")
        consts_loaded = [False]

        def load_consts():
            if consts_loaded[0]:
                return
            consts_loaded[0] = True
            nc.scalar.dma_start(out=bct, in_=bc_[:])
            nc.scalar.dma_start(out=biast2, in_=bias2_[:])

        # --- staging: B9_k[3ci+kh, d', j] = xh[ci, 32k+d', kh*96 + j] ---
        # Only the kh replication is staged (3x bytes, not 8.5x); kd is
        # handled by a free-dim d-offset in the rhs loads. 3 d-chunks with a
        # +2 halo; staging pieces are interleaved with rhs loads so the DMA
        # FIFO never sees a long staging burst.
        DKS = (34, 34, 30)
        b9s = [None] * 3

        def stage(k, pieces=range(9), j0=0, j1=HW27):
            Dk = DKS[k]
            if b9s[k] is None:
                bt = dramp.tile([9, Dk, HW27], F16, tag=f"B9_{k}")
                b9s[k] = bt
            bt = b9s[k]
            for p in pieces:
                ci, kh = p // 3, p % 3
                sap = bass.AP(
                    tensor=xh,
                    offset=ci * S3 + 32 * k * S2 + kh * S + j0,
                    ap=[[S2, Dk], [1, j1 - j0]],
                )
                eng = (nc.sync, nc.scalar, nc.gpsimd)[p % 3]
                eng.dma_start(out=bt[p, :, j0:j1], in_=sap)

        # stage-0 lo half first: the first block only reads j < 3456, so
        # (with range-level dependency tracking) its rhs loads start after
        # ~6us of staging instead of ~15us; the hi half lands right after
        # the first block's rhs loads in the DMA FIFO.
        stage(0, j0=0, j1=3456)
        # exp/sum inputs: needed ~1us after the first conv, so they load
        # right behind the stage-0 lo half (tiny transfers)
        nc.scalar.dma_start(out=biast, in_=bias_[:])
        nc.scalar.dma_start(out=wst, in_=ws_[:])

        pending = None
        es = [None, None]
        for dq0 in (0, 8, 16):
            ndq = min(8, Q - dq0)
            E = 4 * ndq
            kc = dq0 // 8
            for hq0 in (0, 8, 16):
                nq = min(8, Q - hq0)          # 8, 8, 7 h-quads in this chunk
                na1 = nq - 4                  # quads covered by col group a=1
                # rhs im2col tiles [108, E, 384]: row 27g+9kd+3ci+kh reads
                # B9[3ci+kh] at d-offset +kd -> one 9-part DMA per (g, kd)
                # (chunk 0: two E=16 halves from the two staged d-halves)
                rhs = [None, None]
                srcs = [(b9s[kc], DKS[kc], 0, E)]
                for a in (0, 1):
                    t = rhsp.tile([108, 32, 4 * S], F16, tag=f"rhs{a}")
                    rhs[a] = t
                    gs = 4 if a == 0 else na1
                    for bt, Dk, e0, e1 in srcs[:1]:
                        for g in range(gs):
                            hq = hq0 + 4 * a + g
                            for kd in range(3):
                                src = bass.AP(
                                    tensor=bt.tensor,
                                    offset=(bt.offset + kd * HW27
                                            + (4 * hq) * S),
                                    ap=[[Dk * HW27, 9], [HW27, e1 - e0],
                                        [1, 4 * S]],
                                )
                                eng = (nc.sync, nc.scalar, nc.gpsimd)[
                                    (3 * g + kd) % 3]
                                eng.dma_start(
                                    out=t[27 * g + 9 * kd:
                                          27 * g + 9 * kd + 9, e0:e1, :],
                                    in_=src)
                for a in (0, 1):
                    t = rhs[a]
                    gs = 4 if a == 0 else na1
                    for bt, Dk, e0, e1 in srcs[1:]:
                        for g in range(gs):
                            hq = hq0 + 4 * a + g
                            for kd in range(3):
                                src = bass.AP(
                                    tensor=bt.tensor,
                                    offset=(bt.offset + kd * HW27
                                            + (4 * hq) * S),
                                    ap=[[Dk * HW27, 9], [HW27, e1 - e0],
                                        [1, 4 * S]],
                                )
                                eng = (nc.sync, nc.scalar, nc.gpsimd)[
                                    (3 * g + kd) % 3]
                                eng.dma_start(
                                    out=t[27 * g + 9 * kd:
                                          27 * g + 9 * kd + 9, e0:e1, :],
                                    in_=src)
                load_consts()
                if kc == 0 and hq0 == 0:
                    stage(0, j0=3456, j1=HW27)
                if dq0 < 16:
                    # spread next d-chunk's staging: 3 pieces per hq0 block
                    stage(dq0 // 8 + 1, range(3 * (hq0 // 8), 3 * (hq0 // 8) + 3))

                for dq in range(dq0, dq0 + ndq):
                    hp = hpp.tile([128, 4, Q], F32)
                    for jh in (0, 1):
                      Ls = [None] * 2
                      for jl in range(2):
                        j = 2 * jh + jl
                        dsi = 4 * (dq - dq0) + j
                        # --- conv: 3 kw taps x 2 col groups ---
                        L = psl.tile([128, 512], F32)
                        Ls[jl] = L
                        for a in (0, 1):
                            for kw in range(3):
                                ka = 108 if (a == 0 or na1 == 4) else 81
                                r = rhs[a][0:ka, dsi, :].rearrange(
                                    "p (hl w) -> p hl w", hl=4,
                                )[:, :, kw:kw + DU]
                                nc.tensor.matmul(
                                    out=L[64 * a:64 * a + 64, 0:368],
                                    lhsT=wlt[0:ka, kw, :],
                                    rhs=r,
                                    start=(kw == 0),
                                    stop=(kw == 2),
                                    skip_group_check=True,
                                )
                        if stage == 'conv':
                            continue
                        # --- exp(z + b) -> bf16 ---
                        e = ep.tile([128, 368], BF16)
                        es[jl] = e
                        nc.scalar.activation(
                            out=e, in_=L[:, 0:368],
                            func=mybir.ActivationFunctionType.Exp,
                            bias=biast[:, 0:1],
                        )
                      if stage == 'conv':
                        continue
                      # software pipeline: previous half's bcast+pools fire
                      # here, after this half's convs, so the in-order PE
                      # queue never head-blocks on exp/Ln semaphores
                      if pending is not None:
                          pending()
                          pending = None
                      s8 = pss.tile([128, 512], F32)
                      for jl in range(2):
                        # --- group sums, 4x-duplicated rows 32jl..32jl+32 ---
                        nc.tensor.matmul(
                            out=s8[32 * jl:32 * jl + 32, 0:368],
                            lhsT=wst,
                            rhs=es[jl],
                            start=True, stop=True,
                            skip_group_check=True,
                            tile_position=(0, 32 * jl),
                        )
                      if stage == 'sum':
                        continue
                      # --- ln(lambda*S): hi (f16) + optional exact (f32) ---
                      hl16 = hlp.tile([64, 368], F16)
                      nc.scalar.activation(
                          out=hl16, in_=s8[0:64, 0:368],
                          func=mybir.ActivationFunctionType.Ln,
                      )
                      if prec == 'hilo':
                        lnf = lnp.tile([64, 368], F32)
                        nc.scalar.activation(
                            out=lnf, in_=s8[0:64, 0:368],
                            func=mybir.ActivationFunctionType.Ln,
                        )
                        # lo = ln - hi written in place over rows 32jl..+8
                        # (rows +8..16 keep hi; DVE partition bases must be
                        # 32-aligned, and the bc coefficients are the same
                        # for both 8-row halves, so [lo; hi] order is fine)
                        for jl in range(2):
                            nc.vector.tensor_tensor(
                                out=hl16[32 * jl:32 * jl + 8, :],
                                in0=lnf[32 * jl:32 * jl + 8, :],
                                in1=hl16[32 * jl:32 * jl + 8, :],
                                op=mybir.AluOpType.subtract,
                            )
                      if stage == 'ln':
                        continue
                      pending = _mk_tail(nc, prec, bct, biast2, out_, hp,
                                         Ls, hl16, jh, dq, hq0, nq,
                                         finp, outp)
                    if stage in ('conv', 'sum', 'ln'):
                        continue
        if pending is not None:
            pending()
            pending = None


def _build(stage='full', prec='hilo'):
    nc = bacc.Bacc(name="conv_softmax_pool")
    xh = nc.declare_dram_parameter("xh", [CIN, S, S, S], F16, isOutput=False)
    wl_ = nc.declare_dram_parameter("wl", [108, 3, 64], F16, isOutput=False)
    ws_ = nc.declare_dram_parameter("ws", [128, 32], BF16, isOutput=False)
    bc_ = nc.declare_dram_parameter("bc", [128, 128], F16, isOutput=False)
    bias_ = nc.declare_dram_parameter("bias", [128, 1], F32, isOutput=False)
    bias2_ = nc.declare_dram_parameter("bias2", [128, 1], F32, isOutput=False)
    out_ = nc.declare_dram_parameter("out", [COUT, Q, Q, Q], F32, isOutput=True)
    _emit(nc, xh, wl_, ws_, bc_, bias_, bias2_, out_, stage=stage, prec=prec)
    nc.finalize()
    return nc


def _host_prep(w, b):
    """Build lhsT block-diagonal weights and softmax helper matrices."""
    # wl[r, kw, m]: r = 27g + 9ci + 3kd + kh, m = 16g + c  (g = 0..3)
    wh = w.astype(np.float32).astype(np.float16)
    wl = np.zeros((108, 3, 64), np.float16)
    for g in range(4):
        for ci in range(CIN):
            for kd in range(3):
                for kh in range(3):
                    wl[27 * g + 9 * kd + 3 * ci + kh, :, 16 * g:16 * g + 16] = \
                        wh[:, ci, kd, kh, :].T

    # ws32: lambda * group indicator, 4x duplicated along cols so PSUM rows
    # 32j..32j+32 all hold S (rows +8..16 feed the lo computation).
    # lambda = 2^-38 keeps ln's input inside the ACT Ln LUT's valid range.
    lam = np.float32(2.0 ** -38)
    ws = np.zeros((128, 32), np.float32)
    for p in range(128):
        for k in range(32):
            if p // 16 == k % 8:
                ws[p, k] = lam
    # bc: -(group indicator) for the K=16 (hi;lo) subtract matmul,
    # replicated at row offsets 0/32/64/96.
    bc = np.zeros((128, 128), np.float16)
    for j in range(4):
        for k in range(16):
            for p in range(128):
                if p // 16 == k % 8:
                    bc[32 * j + k, p] = -1.0
    bias = np.tile(b.astype(np.float32), 8).reshape(128, 1)
    # y = z - ln(lambda*S) = z - ln S + 38ln2; final exp needs
    # bias2 = b - 38ln2 to recover exp(z + b - ln S).
    bias2 = bias - np.float32(38.0 * np.log(2.0))
    return wl, ws.astype(np.float32), bc, bias, bias2


PREC = "f16"


def kernel(x, w, b):
    key = ("nc", PREC)
    if key not in _cache:
        _cache[key] = _build(prec=PREC)
    nc = _cache[key]

    x = np.asarray(x, np.float32)
    w = np.asarray(w, np.float32)
    b = np.asarray(b, np.float32)
    wl, ws, bc, bias, bias2 = _host_prep(w, b)
    import ml_dtypes
    ws_bf16 = ws.astype(ml_dtypes.bfloat16)

    in_maps = []
    for i in range(N_CORES):
        m = {
            "xh": np.ascontiguousarray(x[i].astype(np.float16)),
            "wl": wl,
            "ws": ws_bf16,
            "bc": bc,
            "bias": bias,
            "bias2": bias2,
        }
        in_maps.append(m)

    res = run_bass_kernel_spmd(nc, in_maps, core_ids=list(range(N_CORES)))
    return np.stack([r["out"] for r in res.results]).astype(np.float32)
